# revision 11
# baseline (speedup 1.0000x reference)
"""GridProjectPooling Trainium2 kernel.

x: (4, 256, 512, 512) f32. h_line/v_line: (31,) f32 grid-line fractions.
Pixels are partitioned into a 32x32 grid of segments (boundaries =
round(512 * sorted(lines))); each grid cell's mean is computed and
broadcast back to all of the cell's pixels. Returns (output, matrix)
exactly like the reference.

Strategy: data-parallel over the 1024 (b,c) images, 128 per NeuronCore.
Per image, everything runs on the PE as one-hot matmuls:
  rowmean(32m, 512w) = ohr_sc.T @ x          (4 accumulating MMs over h-chunks)
  matT(32n, 32m)     = ohc_sc.T @ rowmean.T  (PE transposes + 4 small MMs)
  bmat(32m, 512w)    = matT.T @ ohct         (col broadcast)
  out(h, w)          = ohrt_chunk.T @ bmat   (row broadcast, 4 MMs)
All fp32; memory-bound (2 MB of HBM traffic per image).
"""

import numpy as np
from contextlib import ExitStack

import concourse.bass as bass
import concourse.bacc as bacc
import concourse.mybir as mybir
from concourse import tile
from concourse.bass_utils import run_bass_kernel_spmd

B, C, H, W = 4, 256, 512, 512
NCORES = 8
IMGS = (B * C) // NCORES  # 128 images per core
NCH = H // 128            # 4 partition chunks per image
M = 32                    # row segments
N = 32                    # col segments
CONSTS_F = 1312           # packed const tensor free size
F32 = mybir.dt.float32

_prog_cache = {}


def _build_program(passes=1):
    nc = bacc.Bacc("TRN2", target_bir_lowering=False, debug=False)

    x_d = nc.declare_dram_parameter("x", [IMGS, NCH, 128, W], F32, isOutput=False)
    # all small constants packed into one tensor -> one DMA -> one semaphore
    # layout per partition p: [ohr(4,32) | ohc(4,32) | ohct 512 | ohrt 512 | iden 32]
    consts_d = nc.declare_dram_parameter("consts", [128, CONSTS_F], F32, isOutput=False)
    out_d = nc.declare_dram_parameter("out", [IMGS, NCH, 128, W], F32, isOutput=True)
    mat_d = nc.declare_dram_parameter("mat", [IMGS, M, N], F32, isOutput=True)

    with tile.TileContext(nc) as tc, ExitStack() as ctx:
        consts = ctx.enter_context(tc.tile_pool(name="consts", bufs=1))
        inp = ctx.enter_context(tc.tile_pool(name="inp", bufs=4))
        outp = ctx.enter_context(tc.tile_pool(name="outp", bufs=4))
        small = ctx.enter_context(tc.tile_pool(name="small", bufs=3))
        ps_rm = ctx.enter_context(tc.tile_pool(name="ps_rm", bufs=2, space=bass.MemorySpace.PSUM))
        ps_rmt = ctx.enter_context(tc.tile_pool(name="ps_rmt", bufs=1, space=bass.MemorySpace.PSUM))
        ps_mat = ctx.enter_context(tc.tile_pool(name="ps_mat", bufs=1, space=bass.MemorySpace.PSUM))
        ps_bm = ctx.enter_context(tc.tile_pool(name="ps_bm", bufs=2, space=bass.MemorySpace.PSUM))
        ps_out = ctx.enter_context(tc.tile_pool(name="ps_out", bufs=2, space=bass.MemorySpace.PSUM))

        c_sb = consts.tile([128, CONSTS_F], F32, tag="consts")
        nc.sync.dma_start(c_sb[:], consts_d[:])
        ohr_sb = c_sb[:, 0:128].rearrange("p (c m) -> p c m", c=NCH)
        ohc_sb = c_sb[:, 128:256].rearrange("p (c n) -> p c n", c=NCH)
        ohct_sb = c_sb[0:N, 256:768]
        ohrt_sb = c_sb[0:M, 768:1280]
        iden_sb = c_sb[0:32, 1280:1312]
        mat_acc = consts.tile([M, IMGS, N], F32, tag="mat_acc")

        for i in range(IMGS * passes):
            i = i % IMGS
            x_sb = inp.tile([128, NCH, W], F32, tag="x")
            nc.sync.dma_start(x_sb[:], x_d[i].rearrange("c p w -> p c w"))

            # rowmean[m, w] = sum_h ohr_sc[h, m] * x[h, w]
            rm_ps = ps_rm.tile([M, W], F32, tag="rm")
            for c in range(NCH):
                nc.tensor.matmul(rm_ps[:], ohr_sb[:, c, :], x_sb[:, c, :],
                                 start=(c == 0), stop=(c == NCH - 1))
            rm_sb = small.tile([M, W], F32, tag="rm_sb")
            nc.vector.tensor_copy(rm_sb[:], rm_ps[:])

            # rowmeanT chunks: (128w, 32m) per w-chunk
            rmt_ps = ps_rmt.tile([128, NCH, M], F32, tag="rmt")
            for c in range(NCH):
                nc.tensor.transpose(rmt_ps[:, c, :], rm_sb[:, c * 128:(c + 1) * 128],
                                    iden_sb[:])
            rmt_sb = small.tile([128, NCH, M], F32, tag="rmt_sb")
            nc.vector.tensor_copy(rmt_sb[:], rmt_ps[:])

            # matT[n, m] = sum_w ohc_sc[w, n] * rowmean[m, w]
            mt_ps = ps_mat.tile([N, M], F32, tag="mt")
            for c in range(NCH):
                nc.tensor.matmul(mt_ps[:], ohc_sb[:, c, :], rmt_sb[:, c, :],
                                 start=(c == 0), stop=(c == NCH - 1))
            mt_sb = small.tile([N, M], F32, tag="mt_sb")
            nc.vector.tensor_copy(mt_sb[:], mt_ps[:])
            # matrix output (transposed back) accumulates in SBUF
            nc.vector.transpose(mat_acc[:, i, :], mt_sb[:])

            # bmat[m, w] = matrix[m, col_id[w]]
            bm_ps = ps_bm.tile([M, W], F32, tag="bm")
            nc.tensor.matmul(bm_ps[:], mt_sb[:], ohct_sb[:], start=True, stop=True)
            bm_sb = small.tile([M, W], F32, tag="bm_sb")
            nc.vector.tensor_copy(bm_sb[:], bm_ps[:])

            # out[h, w] = bmat[row_id[h], w], one MM per 128-row chunk
            o_sb = outp.tile([128, NCH, W], F32, tag="o")
            for c in range(NCH):
                o_ps = ps_out.tile([128, W], F32, tag="o_ps")
                nc.tensor.matmul(o_ps[:], ohrt_sb[:, c * 128:(c + 1) * 128], bm_sb[:],
                                 start=True, stop=True)
                if c % 2 == 0:
                    nc.vector.tensor_copy(o_sb[:, c, :], o_ps[:])
                else:
                    nc.scalar.copy(o_sb[:, c, :], o_ps[:])
            nc.sync.dma_start(out_d[i].rearrange("c p w -> p c w"), o_sb[:])

        nc.sync.dma_start(mat_d.rearrange("i m n -> m i n"), mat_acc[:])

    nc.compile()
    return nc


def _pack_consts(oh_r, oh_c, rcount, ccount):
    """Pack all small constants into one (128, CONSTS_F) f32 array."""
    c = np.zeros((128, CONSTS_F), dtype=np.float32)
    # ohr scaled: (H, M) -> (NCH, 128, M) -> partition p gets [c, m] flat
    ohr_sc = (oh_r / rcount[None, :]).reshape(NCH, 128, M)
    c[:, 0:128] = ohr_sc.transpose(1, 0, 2).reshape(128, NCH * M)
    ohc_sc = (oh_c / ccount[None, :]).reshape(NCH, 128, N)
    c[:, 128:256] = ohc_sc.transpose(1, 0, 2).reshape(128, NCH * N)
    c[0:N, 256:768] = oh_c.T
    c[0:M, 768:1280] = oh_r.T
    c[0:32, 1280:1312] = np.eye(32, dtype=np.float32)
    return c


def _segment_ids(frac_lines, size):
    bounds = np.round(size * np.sort(frac_lines)).astype(np.int32)
    return np.searchsorted(bounds, np.arange(size), side="right").astype(np.int32)


def kernel(x, h_line, v_line):
    x = np.ascontiguousarray(x, dtype=np.float32)
    row_id = _segment_ids(np.asarray(h_line, dtype=np.float32), H)
    col_id = _segment_ids(np.asarray(v_line, dtype=np.float32), W)

    oh_r = np.zeros((H, M), dtype=np.float32)
    oh_r[np.arange(H), row_id] = 1.0
    oh_c = np.zeros((W, N), dtype=np.float32)
    oh_c[np.arange(W), col_id] = 1.0
    rcount = np.maximum(oh_r.sum(0), 1.0)
    ccount = np.maximum(oh_c.sum(0), 1.0)

    consts = _pack_consts(oh_r, oh_c, rcount, ccount)

    if "prog" not in _prog_cache:
        _prog_cache["prog"] = _build_program()
    nc = _prog_cache["prog"]

    xs = x.reshape(B * C, H, W)
    in_maps = []
    for k in range(NCORES):
        shard = np.ascontiguousarray(
            xs[k * IMGS:(k + 1) * IMGS].reshape(IMGS, NCH, 128, W))
        in_maps.append({"x": shard, "consts": consts})

    res = run_bass_kernel_spmd(nc, in_maps, list(range(NCORES)))

    output = np.concatenate(
        [res.results[k]["out"].reshape(IMGS, H, W) for k in range(NCORES)]
    ).reshape(B, C, H, W)
    matrix = np.concatenate(
        [res.results[k]["mat"] for k in range(NCORES)]
    ).reshape(B, C, M, N)
    return output, matrix


# revision 23
# speedup vs baseline: 81.1145x; 81.1145x over previous
"""GridProjectPooling Trainium2 kernel.

x: (4, 256, 512, 512) f32. h_line/v_line: (31,) f32 grid-line fractions.
Pixels are partitioned into a 32x32 grid of segments (boundaries =
round(512 * sorted(lines))); each grid cell's mean is computed and
broadcast back to all of the cell's pixels. Returns (output, matrix)
exactly like the reference.

Strategy: data-parallel over the 1024 (b,c) images, 128 per NeuronCore.
Per image, everything runs on the PE as one-hot matmuls:
  rowmean(32m, 512w) = ohr_sc.T @ x          (4 accumulating MMs over h-chunks)
  matT(32n, 32m)     = ohc_sc.T @ rowmean.T  (PE transposes + 4 small MMs)
  bmat(32m, 512w)    = matT.T @ ohct         (col broadcast)
  out(h, w)          = ohrt_chunk.T @ bmat   (row broadcast, 4 MMs)
All fp32; memory-bound (2 MB of HBM traffic per image).
"""

import numpy as np
from contextlib import ExitStack

import concourse.bass as bass
import concourse.bacc as bacc
import concourse.mybir as mybir
from concourse import tile
from concourse.bass_utils import run_bass_kernel_spmd

B, C, H, W = 4, 256, 512, 512
NCORES = 8
IMGS = (B * C) // NCORES  # 128 images per core
NCH = H // 128            # 4 partition chunks per image
M = 32                    # row segments
N = 32                    # col segments
CONSTS_F = 1312           # packed const tensor free size
F32 = mybir.dt.float32

_prog_cache = {}


def _build_program(passes=1):
    nc = bacc.Bacc("TRN2", target_bir_lowering=False, debug=False)

    # image rows are interleaved onto partitions: partition p holds rows
    # 4p+r (r=0..3), so each partition's 4 rows are 8KB contiguous in DRAM
    # and a whole 1MB image moves as a single efficient DMA.
    x_d = nc.declare_dram_parameter("x", [IMGS, 128, NCH, W], F32, isOutput=False)
    # all small constants packed into one tensor -> one DMA -> one semaphore
    # layout per partition p: [ohr(4,32) | ohc(4,32) | ohct 512 | ohrt 512 | iden 32]
    consts_d = nc.declare_dram_parameter("consts", [128, CONSTS_F], F32, isOutput=False)
    out_d = nc.declare_dram_parameter("out", [IMGS, 128, NCH, W], F32, isOutput=True)
    mat_d = nc.declare_dram_parameter("mat", [IMGS, M, N], F32, isOutput=True)

    with tile.TileContext(nc) as tc, ExitStack() as ctx:
        consts = ctx.enter_context(tc.tile_pool(name="consts", bufs=1))
        inp = ctx.enter_context(tc.tile_pool(name="inp", bufs=6))
        outp = ctx.enter_context(tc.tile_pool(name="outp", bufs=6))
        small = ctx.enter_context(tc.tile_pool(name="small", bufs=3))
        ps_rm = ctx.enter_context(tc.tile_pool(name="ps_rm", bufs=1, space=bass.MemorySpace.PSUM))
        ps_rmt = ctx.enter_context(tc.tile_pool(name="ps_rmt", bufs=1, space=bass.MemorySpace.PSUM))
        ps_mat = ctx.enter_context(tc.tile_pool(name="ps_mat", bufs=1, space=bass.MemorySpace.PSUM))
        ps_bm = ctx.enter_context(tc.tile_pool(name="ps_bm", bufs=1, space=bass.MemorySpace.PSUM))
        ps_out = ctx.enter_context(tc.tile_pool(name="ps_out", bufs=1, space=bass.MemorySpace.PSUM))

        c_sb = consts.tile([128, CONSTS_F], F32, tag="consts")
        nc.sync.dma_start(c_sb[:], consts_d[:])
        ohr_sb = c_sb[:, 0:128].rearrange("p (c m) -> p c m", c=NCH)
        ohc_sb = c_sb[:, 128:256].rearrange("p (c n) -> p c n", c=NCH)
        ohct_sb = c_sb[0:N, 256:768]
        ohrt_sb = c_sb[0:M, 768:1280]
        iden_sb = c_sb[0:32, 1280:1312]
        mat_acc = consts.tile([M, IMGS, N], F32, tag="mat_acc")

        for i in range(IMGS * passes):
            i = i % IMGS
            x_sb = inp.tile([128, NCH, W], F32, tag="x")
            nc.sync.dma_start(x_sb[:], x_d[i])

            # rowmean[m, w] = sum_h ohr_sc[h, m] * x[h, w]; partition p of
            # x_sb[:, r, :] is image row 4p+r, ohr_sb[:, r, :] matches.
            rm_ps = ps_rm.tile([M, W], F32, tag="rm")
            for c in range(NCH):
                nc.tensor.matmul(rm_ps[:], ohr_sb[:, c, :], x_sb[:, c, :],
                                 start=(c == 0), stop=(c == NCH - 1))
            rm_sb = small.tile([M, W], F32, tag="rm_sb")
            nc.vector.tensor_copy(rm_sb[:], rm_ps[:])

            # rowmeanT chunks: (128w, 32m) per w-chunk
            rmt_ps = ps_rmt.tile([128, NCH, M], F32, tag="rmt")
            for c in range(NCH):
                nc.tensor.transpose(rmt_ps[:, c, :], rm_sb[:, c * 128:(c + 1) * 128],
                                    iden_sb[:])
            rmt_sb = small.tile([128, NCH, M], F32, tag="rmt_sb")
            nc.vector.tensor_copy(rmt_sb[:], rmt_ps[:])

            # matT[n, m] = sum_w ohc_sc[w, n] * rowmean[m, w]
            mt_ps = ps_mat.tile([N, M], F32, tag="mt")
            for c in range(NCH):
                nc.tensor.matmul(mt_ps[:], ohc_sb[:, c, :], rmt_sb[:, c, :],
                                 start=(c == 0), stop=(c == NCH - 1))
            mt_sb = small.tile([N, M], F32, tag="mt_sb")
            nc.vector.tensor_copy(mt_sb[:], mt_ps[:])
            # matrix output (transposed back) accumulates in SBUF
            nc.vector.transpose(mat_acc[:, i, :], mt_sb[:])

            # bmat[m, w] = matrix[m, col_id[w]]
            bm_ps = ps_bm.tile([M, W], F32, tag="bm")
            nc.tensor.matmul(bm_ps[:], mt_sb[:], ohct_sb[:], start=True, stop=True)
            bm_sb = small.tile([M, W], F32, tag="bm_sb")
            nc.vector.tensor_copy(bm_sb[:], bm_ps[:])

            # out[h, w] = bmat[row_id[h], w]; partition p of slice r is row 4p+r
            o_sb = outp.tile([128, NCH, W], F32, tag="o")
            o_ps = ps_out.tile([128, NCH, W], F32, tag="o_ps")
            for c in range(NCH):
                nc.tensor.matmul(o_ps[:, c, :], ohrt_sb[:, c * 128:(c + 1) * 128],
                                 bm_sb[:], start=True, stop=True)
                if c % 2 == 0:
                    nc.vector.tensor_copy(o_sb[:, c, :], o_ps[:, c, :])
                else:
                    nc.scalar.copy(o_sb[:, c, :], o_ps[:, c, :])
            nc.scalar.dma_start(out_d[i], o_sb[:])

        nc.sync.dma_start(mat_d.rearrange("i m n -> m i n"), mat_acc[:])

    nc.compile()
    return nc


def _pack_consts(oh_r, oh_c, rcount, ccount):
    """Pack all small constants into one (128, CONSTS_F) f32 array.

    Row mapping is interleaved: partition p, variant r <-> image row 4p+r
    (so each partition's rows are contiguous in DRAM). Col mapping stays
    chunked: w-chunk c, partition p <-> column c*128+p.
    """
    c = np.zeros((128, CONSTS_F), dtype=np.float32)
    # ohr scaled: [p, r*32+m] = ohr_sc[4p+r, m]
    ohr_sc = (oh_r / rcount[None, :]).reshape(128, NCH, M)
    c[:, 0:128] = ohr_sc.reshape(128, NCH * M)
    ohc_sc = (oh_c / ccount[None, :]).reshape(NCH, 128, N)
    c[:, 128:256] = ohc_sc.transpose(1, 0, 2).reshape(128, NCH * N)
    c[0:N, 256:768] = oh_c.T
    # ohrt: [m, r*128+p] = oh_r[4p+r, m]
    c[0:M, 768:1280] = oh_r.reshape(128, NCH, M).transpose(2, 1, 0).reshape(M, H)
    c[0:32, 1280:1312] = np.eye(32, dtype=np.float32)
    return c


def _shard(xs, k):
    """Core k's x slice, shaped [IMGS, 128, NCH, W] (row 4p+r on partition p)."""
    return np.ascontiguousarray(
        xs[k * IMGS:(k + 1) * IMGS].reshape(IMGS, 128, NCH, W))


def _segment_ids(frac_lines, size):
    bounds = np.round(size * np.sort(frac_lines)).astype(np.int32)
    return np.searchsorted(bounds, np.arange(size), side="right").astype(np.int32)


def kernel(x, h_line, v_line):
    x = np.ascontiguousarray(x, dtype=np.float32)
    row_id = _segment_ids(np.asarray(h_line, dtype=np.float32), H)
    col_id = _segment_ids(np.asarray(v_line, dtype=np.float32), W)

    oh_r = np.zeros((H, M), dtype=np.float32)
    oh_r[np.arange(H), row_id] = 1.0
    oh_c = np.zeros((W, N), dtype=np.float32)
    oh_c[np.arange(W), col_id] = 1.0
    rcount = np.maximum(oh_r.sum(0), 1.0)
    ccount = np.maximum(oh_c.sum(0), 1.0)

    consts = _pack_consts(oh_r, oh_c, rcount, ccount)

    if "prog" not in _prog_cache:
        _prog_cache["prog"] = _build_program()
    nc = _prog_cache["prog"]

    xs = x.reshape(B * C, H, W)
    in_maps = [{"x": _shard(xs, k), "consts": consts} for k in range(NCORES)]

    res = run_bass_kernel_spmd(nc, in_maps, list(range(NCORES)))

    output = np.concatenate(
        [res.results[k]["out"].reshape(IMGS, H, W) for k in range(NCORES)]
    ).reshape(B, C, H, W)
    matrix = np.concatenate(
        [res.results[k]["mat"] for k in range(NCORES)]
    ).reshape(B, C, M, N)
    return output, matrix
